# revision 1
# baseline (speedup 1.0000x reference)
"""Causal multi-head attention on 8 trn2 NeuronCores.

Sharding: core c -> (batch b = c//2, head-group hg = c%2).
Each head-group owns 8 of the 16 heads (512 of the 1024 embed dims after
the head split). Per core:
  - qT, kT   = (x[b] @ Wq_hg)^T / 8, (x[b] @ Wk_hg)^T      [cols, rows]
  - v        = x[b] @ Wv_hg                                 [rows, cols]
  - scoresT  = kT.T-contract: [krows, qrows] tiles; exp; causal mask
  - ctxT_un  = [V | 1] ^T-style augmented matmul -> [65, qrows] with the
               softmax denominator in row 64 (no transposes anywhere)
  - ctxT     = ctxT_un * (1/denom)  (denom broadcast via K=1 ones matmul)
  - partial  = ctxT.T @ Wproj_hg rows (+ bproj on hg==0 cores only)
Host: out[b] = partial(b,0) + partial(b,1).

All matmuls run as float32r (fp32 stored, FP22 multiply) for 4x PE rate
vs true fp32.
"""

import sys

try:
    import concourse.bass as bass  # noqa: F401
except Exception:
    sys.path.insert(0, "/opt/trn_rl_repo")

import numpy as np

import concourse.bass as bass
import concourse.mybir as mybir
import concourse.tile as tile
from concourse import bacc
from concourse.bass_utils import run_bass_kernel_spmd

F32 = mybir.dt.float32
F32R = mybir.dt.float32r
AF = mybir.ActivationFunctionType

B, S, E = 4, 1024, 1024
H, D = 16, 64
HG = 2              # head groups (cores per batch)
HPG = H // HG       # 8 heads per group
EG = HPG * D        # 512 embed cols per group
P = 128
ET = E // P         # 8 embed tiles
RT = S // P         # 8 row tiles
CT = EG // P        # 4 col tiles of the group's q/k
QCH = 512           # q-chunk (moving free dim)
NQC = S // QCH      # 2 q chunks
SCALE = 1.0 / np.sqrt(D)


def _emit(nc, tc, with_bias):
    xT = nc.dram_tensor("xT", [E, S], F32R, kind="ExternalInput")
    wq = nc.dram_tensor("wq", [E, EG], F32R, kind="ExternalInput")
    wk = nc.dram_tensor("wk", [E, EG], F32R, kind="ExternalInput")
    wv = nc.dram_tensor("wv", [E, EG], F32R, kind="ExternalInput")
    wp = nc.dram_tensor("wp", [EG, E], F32R, kind="ExternalInput")
    bq = nc.dram_tensor("bq", [P, CT], F32, kind="ExternalInput")
    bk = nc.dram_tensor("bk", [P, CT], F32, kind="ExternalInput")
    bv = nc.dram_tensor("bv", [1, EG], F32R, kind="ExternalInput")
    bp = nc.dram_tensor("bp", [1, E], F32R, kind="ExternalInput")
    mask = nc.dram_tensor("mask", [P, P], F32, kind="ExternalInput")
    ones_in = nc.dram_tensor("ones", [1, QCH], F32R, kind="ExternalInput")
    vones_in = nc.dram_tensor("vones", [P, HPG], F32R, kind="ExternalInput")
    out = nc.dram_tensor("out", [S, E], F32, kind="ExternalOutput")


    with (
        tc.tile_pool(name="xt", bufs=1) as p_xt,
        tc.tile_pool(name="wqkv", bufs=1) as p_w,
        tc.tile_pool(name="wp", bufs=1) as p_wp,
        tc.tile_pool(name="qt", bufs=1) as p_qt,
        tc.tile_pool(name="kt", bufs=1) as p_kt,
        tc.tile_pool(name="vaug", bufs=1) as p_va,
        tc.tile_pool(name="ctxT", bufs=1) as p_ctx,
        tc.tile_pool(name="exps", bufs=6) as p_exp,
        tc.tile_pool(name="small", bufs=1) as p_sm,
        tc.tile_pool(name="recip", bufs=4) as p_rc,
        tc.tile_pool(name="bcsb", bufs=2) as p_bc,
        tc.tile_pool(name="osb", bufs=3) as p_osb,
        tc.tile_pool(name="mm", bufs=3, space="PSUM") as p_mm,
        tc.tile_pool(name="av", bufs=3, space="PSUM") as p_av,
        tc.tile_pool(name="bc", bufs=2, space="PSUM") as p_pbc,
    ):
        # ---- small constants ----
        ones_sb = p_sm.tile([1, QCH], F32R, tag="ones", name="ones")
        nc.sync.dma_start(ones_sb[:], ones_in[:])
        vones_sb = p_sm.tile([P, HPG], F32R, tag="vones", name="vones")
        nc.sync.dma_start(vones_sb[:], vones_in[:])
        mask_sb = p_sm.tile([P, P], F32, tag="mask", name="maskt")
        nc.sync.dma_start(mask_sb[:], mask[:])
        bq_sb = p_sm.tile([P, CT], F32, tag="bq", name="bqt")
        nc.sync.dma_start(bq_sb[:], bq[:])
        bk_sb = p_sm.tile([P, CT], F32, tag="bk", name="bkt")
        nc.sync.dma_start(bk_sb[:], bk[:])
        bv_sb = p_sm.tile([1, EG], F32R, tag="bv", name="bvt")
        nc.sync.dma_start(bv_sb[:], bv[:])
        bp_sb = p_sm.tile([1, E], F32R, tag="bp", name="bpt")
        nc.sync.dma_start(bp_sb[:], bp[:])

        # ---- input loads, split into ~64-128KB chunks and ordered by
        # first use so the first QKV groups start within a few us ----
        xt_t = [p_xt.tile([P, S], F32R, tag=f"xt{et}", name=f"xt{et}")
                for et in range(ET)]
        wq_t = [p_w.tile([P, EG], F32R, tag=f"wq{et}", name=f"wq{et}")
                for et in range(ET)]
        wk_t = [p_w.tile([P, EG], F32R, tag=f"wk{et}", name=f"wk{et}")
                for et in range(ET)]
        wv_t = [p_w.tile([P, EG], F32R, tag=f"wv{et}", name=f"wv{et}")
                for et in range(ET)]
        wp_t = [p_wp.tile([P, E], F32R, tag=f"wp{et}", name=f"wp{et}")
                for et in range(EG // P)]
        XC = S // 4      # 256-col x chunks

        def load_x(c):
            for et in range(ET):
                nc.sync.dma_start(
                    xt_t[et][:, c * XC:(c + 1) * XC],
                    xT[et * P:(et + 1) * P, c * XC:(c + 1) * XC])

        def load_w(lst, srt, ct):
            for et in range(ET):
                nc.sync.dma_start(
                    lst[et][:, ct * P:(ct + 1) * P],
                    srt[et * P:(et + 1) * P, ct * P:(ct + 1) * P])

        load_x(0)
        load_w(wq_t, wq, 0)
        load_x(1)
        load_w(wk_t, wk, 0)
        load_x(2)
        load_x(3)
        for ct in range(1, CT):
            load_w(wq_t, wq, ct)
            load_w(wk_t, wk, ct)
        for half in range(2):
            for et in range(ET):
                nc.sync.dma_start(
                    wv_t[et][:, half * 256:(half + 1) * 256],
                    wv[et * P:(et + 1) * P, half * 256:(half + 1) * 256])
        for et in range(EG // P):
            for half in range(2):
                nc.sync.dma_start(
                    wp_t[et][:, half * QCH:(half + 1) * QCH],
                    wp[et * P:(et + 1) * P, half * QCH:(half + 1) * QCH])

        # ---- QKV projection ----
        # qT/kT: [cols, rows] = W_slice.T-contract over embed.
        qT_t = [p_qt.tile([P, S], F32R, tag=f"qt{ct}", name=f"qt{ct}") for ct in range(CT)]
        kT_t = [p_kt.tile([P, S], F32R, tag=f"kt{ct}", name=f"kt{ct}") for ct in range(CT)]
        for ct in range(CT):
            for rc in range(NQC):
                for dst, wt, bias, scl in (
                        (qT_t, wq_t, bq_sb, float(SCALE)),
                        (kT_t, wk_t, bk_sb, 1.0),
                ):
                    ps = p_mm.tile([P, QCH], F32, tag="mm", name="mm")
                    for et in range(ET):
                        nc.tensor.matmul(
                            ps[:],
                            wt[et][:, ct * P:(ct + 1) * P],
                            xt_t[et][:, rc * QCH:(rc + 1) * QCH],
                            start=(et == 0), stop=(et == ET - 1),
                        )
                    # out = in*scale + bias  (bias pre-scaled on host for q)
                    nc.scalar.activation(
                        dst[ct][:, rc * QCH:(rc + 1) * QCH], ps[:],
                        AF.Identity, bias=bias[:, ct:ct + 1], scale=scl,
                    )

        # v natural [rows, cols], packed into vaug [rows, 8*(64+1)] with a
        # ones column per head for the softmax denominator.
        va_t = []
        for rt in range(RT):
            va = p_va.tile([P, HPG * (D + 1)], F32R, tag=f"va{rt}", name=f"va{rt}")
            va3 = va[:].rearrange("p (h d) -> p h d", h=HPG)
            nc.vector.tensor_copy(va3[:, :, D:D + 1], vones_sb[:].rearrange("p (h o) -> p h o", o=1))
            ps = p_mm.tile([P, EG], F32, tag="mm", name="mm")
            for et in range(ET):
                nc.tensor.matmul(
                    ps[:],
                    xt_t[et][:, rt * P:(rt + 1) * P],
                    wv_t[et][:],
                    start=(et == 0),
                    stop=(et == ET - 1 and not with_bias),
                )
            if with_bias:
                nc.tensor.matmul(
                    ps[:], ones_sb[0:1, 0:P], bv_sb[0:1, :],
                    start=False, stop=True,
                )
            for h in range(HPG):
                nc.vector.tensor_copy(
                    va3[:, h, 0:D], ps[:, h * D:(h + 1) * D])
            va_t.append(va)

        # ---- attention (per head, per q-chunk) ----
        ctx_t = [p_ctx.tile([P, S], F32R, tag=f"cx{i}", name=f"cx{i}") for i in range(CT)]
        for h in range(HPG):
            hp, hb = h // 2, (h % 2) * D     # tile index / partition base
            for qc in range(NQC):
                av = p_av.tile([D + 1, QCH], F32, tag="av", name="av")
                n_kt = (qc + 1) * (QCH // P)
                for kt in range(n_kt):
                    off = max(0, (kt - qc * (QCH // P))) * P
                    n = QCH - off
                    sc = p_mm.tile([P, QCH], F32, tag="mm", name="mm")
                    nc.tensor.matmul(
                        sc[:, 0:n],
                        kT_t[hp][hb:hb + D, kt * P:(kt + 1) * P],
                        qT_t[hp][hb:hb + D, qc * QCH + off:(qc + 1) * QCH],
                        start=True, stop=True,
                        tile_position=(hb, 0),
                    )
                    ex = p_exp.tile([P, QCH], F32R, tag="ex", name="ex")
                    nc.scalar.activation(ex[:, 0:n], sc[:, 0:n], AF.Exp)
                    # diagonal block (global q-tile index == kt): mask
                    # the first P columns of the exp'd tile
                    if kt >= qc * (QCH // P):
                        nc.vector.tensor_mul(
                            ex[:, 0:P], ex[:, 0:P], mask_sb[:])
                    nc.tensor.matmul(
                        av[:, off:QCH],
                        va_t[kt][:].rearrange("p (h d) -> p h d", h=HPG)[:, h, :],
                        ex[:, 0:n],
                        start=(kt == 0), stop=(kt == n_kt - 1),
                    )
                rc_sb = p_rc.tile([1, QCH], F32R, tag="rc", name="rc")
                nc.vector.reciprocal(rc_sb[:], av[D:D + 1, :])
                bcp = p_pbc.tile([D, QCH], F32, tag="bc", name="bc")
                nc.tensor.matmul(
                    bcp[:], ones_sb[0:1, 0:D], rc_sb[:],
                    start=True, stop=True,
                )
                bc_sb = p_bc.tile([D, QCH], F32, tag="bcs", name="bcs")
                nc.vector.tensor_copy(bc_sb[:], bcp[:])
                nc.vector.tensor_mul(
                    ctx_t[hp][hb:hb + D, qc * QCH:(qc + 1) * QCH],
                    av[0:D, :], bc_sb[:])

        # ---- output projection: partial = ctxT.T @ wp (+ bp) ----
        for rt in range(RT):
            for cc in range(E // QCH):
                ps = p_mm.tile([P, QCH], F32, tag="mm", name="mm")
                for et in range(EG // P):
                    nc.tensor.matmul(
                        ps[:],
                        ctx_t[et][:, rt * P:(rt + 1) * P],
                        wp_t[et][:, cc * QCH:(cc + 1) * QCH],
                        start=(et == 0),
                        stop=(et == EG // P - 1 and not with_bias),
                    )
                if with_bias:
                    nc.tensor.matmul(
                        ps[:], ones_sb[0:1, 0:P],
                        bp_sb[0:1, cc * QCH:(cc + 1) * QCH],
                        start=False, stop=True,
                    )
                osb = p_osb.tile([P, QCH], F32, tag="osb", name="osb")
                nc.vector.tensor_copy(osb[:], ps[:])
                for half in range(2):
                    nc.sync.dma_start(
                        out[rt * P:(rt + 1) * P,
                            cc * QCH + half * 256:cc * QCH + (half + 1) * 256],
                        osb[:, half * 256:(half + 1) * 256])


def build_nc(with_bias=False):
    nc = bacc.Bacc("TRN2", target_bir_lowering=False, debug=False)
    with tile.TileContext(nc) as tc, nc.allow_low_precision(
        reason="fp32r (FP22-multiply) matmul pipeline; fp32 accumulate"
    ):
        _emit(nc, tc, with_bias)
    nc.compile()
    return nc


def make_in_maps(x, Wqkv, bqkv, Wproj, bproj):
    x = np.asarray(x, dtype=np.float32)
    Wqkv = np.asarray(Wqkv, dtype=np.float32)
    bqkv = np.asarray(bqkv, dtype=np.float32)
    Wproj = np.asarray(Wproj, dtype=np.float32)
    bproj = np.asarray(bproj, dtype=np.float32)
    mask = np.triu(np.ones((P, P), dtype=np.float32))  # [k, q]: k <= q
    in_maps = []
    for c in range(8):
        b, hg = c // 2, c % 2
        g = slice(hg * EG, (hg + 1) * EG)
        in_maps.append({
            "xT": np.ascontiguousarray(x[b].T),
            "wq": np.ascontiguousarray(Wqkv[:, 0 * E:1 * E][:, g]),
            "wk": np.ascontiguousarray(Wqkv[:, 1 * E:2 * E][:, g]),
            "wv": np.ascontiguousarray(Wqkv[:, 2 * E:3 * E][:, g]),
            "wp": np.ascontiguousarray(Wproj[g, :]),
            "bq": np.ascontiguousarray(
                (bqkv[0 * E:1 * E][g] * SCALE).reshape(CT, P).T),
            "bk": np.ascontiguousarray(
                bqkv[1 * E:2 * E][g].reshape(CT, P).T),
            "bv": bqkv[2 * E:3 * E][g].reshape(1, EG).copy(),
            "bp": (bproj if hg == 0 else np.zeros_like(bproj)).reshape(1, E).copy(),
            "mask": mask,
            "ones": np.ones((1, QCH), dtype=np.float32),
            "vones": np.ones((P, HPG), dtype=np.float32),
        })
    return in_maps


def kernel(x, Wqkv, bqkv, Wproj, bproj):
    with_bias = bool(
        np.any(np.asarray(bqkv)) or np.any(np.asarray(bproj)))
    nc = build_nc(with_bias)
    in_maps = make_in_maps(x, Wqkv, bqkv, Wproj, bproj)
    res = run_bass_kernel_spmd(nc, in_maps, list(range(8))).results
    out = np.zeros((B, S, E), dtype=np.float32)
    for c in range(8):
        out[c // 2] += res[c]["out"]
    return out



# revision 4
# speedup vs baseline: 1.5982x; 1.5982x over previous
"""Causal multi-head attention on 8 trn2 NeuronCores.

Sharding: core c -> (batch b = c//2, head-group hg = c%2).
Each head-group owns 8 of the 16 heads (512 of the 1024 embed dims after
the head split).

v2 layout (all matmul operands bf16, PSUM accumulation fp32):
  - qT, kT = (x[b] @ Wq_hg)^T, (x[b] @ Wk_hg)^T    [cols, rows] bf16
    (softmax 1/sqrt(d) scale folded into Wq on host)
  - v packed as va [rows, 8*(64+1)] bf16 with a ones column per head so
    the attn@V matmul also produces the softmax denominator (row 64).
  - scoresT [k, q] per (head, 512-q-chunk, 128-k-tile); the causal mask
    is ADDED in PSUM via an extra identity x (-1e9 upper-tri) matmul on
    diagonal tiles, then a single exp -> bf16.
  - normalize: denom row -> reciprocal_approx_fast (DVE) ->
    partition_broadcast (GPSIMD) -> one DVE multiply into ctxT bf16.
  - partial = ctxT.T @ Wproj_hg (+ bproj on hg==0 cores).
Host: out[b] = partial(b,0) + partial(b,1).

Emission order is software-pipelined: score matmuls run 3 k-tiles ahead
of the attn@V matmuls (so PE never waits on exp), and the q/k
projections for later head-groups are interleaved into the attention
stream to keep the PE HAM clock-gate at full speed.
"""

import sys

try:
    import concourse.bass as bass  # noqa: F401
except Exception:
    sys.path.insert(0, "/opt/trn_rl_repo")

import ml_dtypes
import numpy as np

import concourse.bass as bass
import concourse.mybir as mybir
import concourse.tile as tile
from concourse import bacc
from concourse.bass_utils import run_bass_kernel_spmd

F32 = mybir.dt.float32
F32R = mybir.dt.float32r
BF16 = mybir.dt.bfloat16
AF = mybir.ActivationFunctionType
BF = ml_dtypes.bfloat16

B, S, E = 4, 1024, 1024
H, D = 16, 64
HG = 2              # head groups (cores per batch)
HPG = H // HG       # 8 heads per group
EG = HPG * D        # 512 embed cols per group
P = 128
ET = E // P         # 8 embed tiles
RT = S // P         # 8 row tiles
CT = EG // P        # 4 col tiles of the group's q/k
QCH = 512           # q-chunk (moving free dim)
NQC = S // QCH      # 2 q chunks
KTQ = QCH // P      # 4 k-tiles per q chunk
SCALE = 1.0 / np.sqrt(D)
NEG = -1e9


def _emit(nc, tc, with_bias):
    xT = nc.dram_tensor("xT", [E, S], BF16, kind="ExternalInput")
    wq = nc.dram_tensor("wq", [E, EG], BF16, kind="ExternalInput")
    wk = nc.dram_tensor("wk", [E, EG], BF16, kind="ExternalInput")
    wv = nc.dram_tensor("wv", [E, EG], BF16, kind="ExternalInput")
    wp = nc.dram_tensor("wp", [EG, E], BF16, kind="ExternalInput")
    bq = nc.dram_tensor("bq", [P, CT], F32, kind="ExternalInput")
    bk = nc.dram_tensor("bk", [P, CT], F32, kind="ExternalInput")
    bv = nc.dram_tensor("bv", [1, EG], F32R, kind="ExternalInput")
    bp = nc.dram_tensor("bp", [1, E], F32R, kind="ExternalInput")
    bmask = nc.dram_tensor("bmask", [P, P], BF16, kind="ExternalInput")
    ident = nc.dram_tensor("ident", [P, P], BF16, kind="ExternalInput")
    ones_in = nc.dram_tensor("ones", [1, QCH], F32R, kind="ExternalInput")
    vones_in = nc.dram_tensor("vones", [P, HPG], BF16, kind="ExternalInput")
    out = nc.dram_tensor("out", [S, E], F32, kind="ExternalOutput")

    with (
        tc.tile_pool(name="big", bufs=1) as p_big,
        tc.tile_pool(name="exs", bufs=6) as p_ex,
        tc.tile_pool(name="rc", bufs=4) as p_rc,
        tc.tile_pool(name="rcb", bufs=4) as p_rcb,
        tc.tile_pool(name="osb", bufs=4) as p_osb,
        tc.tile_pool(name="sm", bufs=1) as p_sm,
        tc.tile_pool(name="sc", bufs=3, space="PSUM") as p_sc,
        tc.tile_pool(name="qk", bufs=2, space="PSUM") as p_qk,
        tc.tile_pool(name="avp", bufs=3, space="PSUM") as p_av,
    ):
        # ---- small constants (loaded first) ----
        ones_sb = p_sm.tile([1, QCH], F32R, tag="ones", name="ones")
        nc.sync.dma_start(ones_sb[:], ones_in[:])
        vones_sb = p_sm.tile([P, HPG], BF16, tag="vones", name="vones")
        nc.sync.dma_start(vones_sb[:], vones_in[:])
        bmask_sb = p_sm.tile([P, P], BF16, tag="bmask", name="bmaskt")
        nc.sync.dma_start(bmask_sb[:], bmask[:])
        ident_sb = p_sm.tile([P, P], BF16, tag="ident", name="identt")
        nc.sync.dma_start(ident_sb[:], ident[:])
        bq_sb = p_sm.tile([P, CT], F32, tag="bq", name="bqt")
        nc.sync.dma_start(bq_sb[:], bq[:])
        bk_sb = p_sm.tile([P, CT], F32, tag="bk", name="bkt")
        nc.sync.dma_start(bk_sb[:], bk[:])
        bv_sb = p_sm.tile([1, EG], F32R, tag="bv", name="bvt")
        nc.sync.dma_start(bv_sb[:], bv[:])
        bp_sb = p_sm.tile([1, E], F32R, tag="bp", name="bpt")
        nc.sync.dma_start(bp_sb[:], bp[:])

        # ---- persistent sbuf tiles ----
        xt_t = [p_big.tile([P, S], BF16, tag=f"xt{et}", name=f"xt{et}")
                for et in range(ET)]
        wq_t = [p_big.tile([P, EG], BF16, tag=f"wq{et}", name=f"wq{et}")
                for et in range(ET)]
        wk_t = [p_big.tile([P, EG], BF16, tag=f"wk{et}", name=f"wk{et}")
                for et in range(ET)]
        wv_t = [p_big.tile([P, EG], BF16, tag=f"wv{et}", name=f"wv{et}")
                for et in range(ET)]
        wp_t = [p_big.tile([P, E], BF16, tag=f"wp{et}", name=f"wp{et}")
                for et in range(CT)]
        qT_t = [p_big.tile([P, S], BF16, tag=f"qt{ct}", name=f"qt{ct}")
                for ct in range(CT)]
        kT_t = [p_big.tile([P, S], BF16, tag=f"kt{ct}", name=f"kt{ct}")
                for ct in range(CT)]
        va_t = [p_big.tile([P, HPG * (D + 1)], BF16, tag=f"va{rt}",
                           name=f"va{rt}") for rt in range(RT)]
        ctx_t = [p_big.tile([P, S], BF16, tag=f"cx{ct}", name=f"cx{ct}")
                 for ct in range(CT)]

        # ---- input DMA, ordered by first use ----
        # interleaved xt(first half)/wq/wk so q-ct0 then k-ct0 can start
        # within ~1us; then wv for the v projection, the xt second halves
        # (needed by v rt>=4), and wp last.
        for et in range(ET):
            nc.sync.dma_start(xt_t[et][:, 0:QCH], xT[et * P:(et + 1) * P, 0:QCH])
            nc.sync.dma_start(wq_t[et][:], wq[et * P:(et + 1) * P, :])
            nc.sync.dma_start(wk_t[et][:], wk[et * P:(et + 1) * P, :])
        for et in range(ET):
            nc.sync.dma_start(wv_t[et][:], wv[et * P:(et + 1) * P, :])
        for et in range(ET):
            nc.sync.dma_start(xt_t[et][:, QCH:S], xT[et * P:(et + 1) * P, QCH:S])
        for et in range(CT):
            nc.sync.dma_start(wp_t[et][:], wp[et * P:(et + 1) * P, :])

        # ---- q/k projection chunk: qT/kT[ct][:, rc*QCH:+QCH] ----
        def emit_qk_chunk(dst, w_t, b_sb, ct, rc, engine):
            ps = p_qk.tile([P, QCH], F32, tag="qk", name="qk")
            for et in range(ET):
                nc.tensor.matmul(
                    ps[:],
                    w_t[et][:, ct * P:(ct + 1) * P],
                    xt_t[et][:, rc * QCH:(rc + 1) * QCH],
                    start=(et == 0), stop=(et == ET - 1),
                )
            dst_ap = dst[ct][:, rc * QCH:(rc + 1) * QCH]
            if with_bias:
                nc.scalar.activation(
                    dst_ap, ps[:], AF.Identity, bias=b_sb[:, ct:ct + 1])
            elif engine == "act":
                nc.scalar.activation(dst_ap, ps[:], AF.Copy)
            else:
                nc.vector.tensor_copy(dst_ap, ps[:])

        # ---- v projection + augmented-va pack for one row tile ----
        def emit_v_rt(rt):
            va3 = va_t[rt][:].rearrange("p (h d) -> p h d", h=HPG)
            nc.vector.tensor_copy(
                va3[:, :, D:D + 1],
                vones_sb[:].rearrange("p (h o) -> p h o", o=1))
            ps = p_qk.tile([P, QCH], F32, tag="qk", name="qk")
            for et in range(ET):
                nc.tensor.matmul(
                    ps[:, 0:EG],
                    xt_t[et][:, rt * P:(rt + 1) * P],
                    wv_t[et][:],
                    start=(et == 0),
                    stop=(et == ET - 1 and not with_bias),
                )
            if with_bias:
                nc.tensor.matmul(
                    ps[:, 0:EG], ones_sb[0:1, 0:P], bv_sb[0:1, :],
                    start=False, stop=True,
                )
            ps3 = ps[:, 0:EG].rearrange("p (h d) -> p h d", h=HPG)
            nc.scalar.activation(va3[:, :, 0:D], ps3[:], AF.Copy)

        # ---- lead phase: q-ct0, k-ct0, v ----
        emit_qk_chunk(qT_t, wq_t, bq_sb, 0, 0, "act")
        emit_qk_chunk(qT_t, wq_t, bq_sb, 0, 1, "act")
        emit_qk_chunk(kT_t, wk_t, bk_sb, 0, 0, "act")
        emit_qk_chunk(kT_t, wk_t, bk_sb, 0, 1, "act")
        for rt in range(RT):
            emit_v_rt(rt)

        # remaining q/k chunks, injected mid-attention (PE work to keep
        # the HAM clock-gate warm; converts ride the DVE queue)
        inject = []
        for ct in range(1, CT):
            inject.append((qT_t, wq_t, bq_sb, ct, 0))
            inject.append((kT_t, wk_t, bk_sb, ct, 0))
            inject.append((qT_t, wq_t, bq_sb, ct, 1))
            inject.append((kT_t, wk_t, bk_sb, ct, 1))
        inject.reverse()  # pop() from the front of the schedule

        # ---- attention ----
        # unit = (head, q-chunk). Per unit: n_kt score matmuls (k-tiles)
        # with the causal-diagonal mask added in PSUM, exp -> bf16, attn@V
        # accumulation into av (denominator rides in row D), then
        # recip -> broadcast -> multiply into ctxT.
        for h in range(HPG):
            hp, hb = h // 2, (h % 2) * D
            va3s = [va_t[kt][:].rearrange("p (h d) -> p h d", h=HPG)[:, h, :]
                    for kt in range(RT)]
            for qc in range(NQC):
                n_kt = (qc + 1) * KTQ
                av = p_av.tile([D + 1, QCH], F32, tag="av", name="av")
                exs = {}

                def emit_sc(kt, qc=qc, exs=exs):
                    off = max(0, kt - qc * KTQ) * P
                    n = QCH - off
                    diag = (qc == 0) or (kt >= KTQ)
                    sc = p_sc.tile([P, QCH], F32, tag="sc", name="sc")
                    nc.tensor.matmul(
                        sc[:, 0:n],
                        kT_t[hp][hb:hb + D, kt * P:(kt + 1) * P],
                        qT_t[hp][hb:hb + D,
                                 qc * QCH + off:(qc + 1) * QCH],
                        start=True, stop=not diag,
                        tile_position=(hb, 0),
                    )
                    if diag:
                        nc.tensor.matmul(
                            sc[:, 0:P], ident_sb[:], bmask_sb[:],
                            start=False, stop=True, skip_group_check=True,
                        )
                    ex = p_ex.tile([P, QCH], BF16, tag="ex", name="ex")
                    nc.scalar.activation(ex[:, 0:n], sc[:, 0:n], AF.Exp)
                    exs[kt] = (ex, off, n)

                LOOK = 3
                for kt in range(min(LOOK, n_kt)):
                    emit_sc(kt)
                # inject one q/k projection chunk mid-unit: the queued
                # exps keep the Act engine busy while PE runs it
                if inject:
                    emit_qk_chunk(*inject.pop(), "dve")
                for kt in range(n_kt):
                    ex, off, n = exs.pop(kt)
                    nc.tensor.matmul(
                        av[:, off:QCH],
                        va3s[kt],
                        ex[:, 0:n],
                        start=(kt == 0), stop=(kt == n_kt - 1),
                    )
                    if kt + LOOK < n_kt:
                        emit_sc(kt + LOOK)

                # normalize: all off the PE stream
                # (reciprocal_approx_fast's bitwise seed misreads PSUM,
                # so stage the denominator row through SBUF first)
                dn_sb = p_rc.tile([1, QCH], F32, tag="dn", name="dn")
                nc.vector.tensor_copy(dn_sb[:], av[D:D + 1, :])
                rc_sb = p_rc.tile([1, QCH], F32, tag="rc", name="rc")
                nc.vector.reciprocal_approx_fast(rc_sb[:], dn_sb[:])
                rcb = p_rcb.tile([D, QCH], F32, tag="rcb", name="rcb")
                nc.gpsimd.partition_broadcast(rcb[:], rc_sb[:], channels=D)
                nc.vector.tensor_mul(
                    ctx_t[hp][hb:hb + D, qc * QCH:(qc + 1) * QCH],
                    av[0:D, :], rcb[:])

        # ---- output projection: partial = ctxT.T @ wp (+ bp) ----
        osb_eng = 0
        for rt in range(RT):
            for cc in range(E // QCH):
                ps = p_sc.tile([P, QCH], F32, tag="sc", name="sc")
                for et in range(CT):
                    nc.tensor.matmul(
                        ps[:],
                        ctx_t[et][:, rt * P:(rt + 1) * P],
                        wp_t[et][:, cc * QCH:(cc + 1) * QCH],
                        start=(et == 0),
                        stop=(et == CT - 1 and not with_bias),
                    )
                if with_bias:
                    nc.tensor.matmul(
                        ps[:], ones_sb[0:1, 0:P],
                        bp_sb[0:1, cc * QCH:(cc + 1) * QCH],
                        start=False, stop=True,
                    )
                osb = p_osb.tile([P, QCH], F32, tag="osb", name="osb")
                if osb_eng == 0:
                    nc.vector.tensor_copy(osb[:], ps[:])
                else:
                    nc.scalar.activation(osb[:], ps[:], AF.Copy)
                osb_eng = (osb_eng + 1) % 2
                nc.sync.dma_start(
                    out[rt * P:(rt + 1) * P, cc * QCH:(cc + 1) * QCH],
                    osb[:])


def build_nc(with_bias=False):
    nc = bacc.Bacc("TRN2", target_bir_lowering=False, debug=False)
    with tile.TileContext(nc) as tc, nc.allow_low_precision(
        reason="bf16 matmul pipeline; fp32 PSUM accumulate"
    ):
        _emit(nc, tc, with_bias)
    nc.compile()
    return nc


def make_in_maps(x, Wqkv, bqkv, Wproj, bproj):
    x = np.asarray(x, dtype=np.float32)
    Wqkv = np.asarray(Wqkv, dtype=np.float32)
    bqkv = np.asarray(bqkv, dtype=np.float32)
    Wproj = np.asarray(Wproj, dtype=np.float32)
    bproj = np.asarray(bproj, dtype=np.float32)
    keep = np.triu(np.ones((P, P), dtype=np.float32))  # [k, q]: k <= q
    bmask = np.where(keep > 0, 0.0, NEG).astype(BF)
    ident = np.eye(P, dtype=np.float32).astype(BF)
    in_maps = []
    for c in range(8):
        b, hg = c // 2, c % 2
        g = slice(hg * EG, (hg + 1) * EG)
        in_maps.append({
            "xT": np.ascontiguousarray(x[b].T).astype(BF),
            "wq": np.ascontiguousarray(
                Wqkv[:, 0 * E:1 * E][:, g] * SCALE).astype(BF),
            "wk": np.ascontiguousarray(Wqkv[:, 1 * E:2 * E][:, g]).astype(BF),
            "wv": np.ascontiguousarray(Wqkv[:, 2 * E:3 * E][:, g]).astype(BF),
            "wp": np.ascontiguousarray(Wproj[g, :]).astype(BF),
            "bq": np.ascontiguousarray(
                (bqkv[0 * E:1 * E][g] * SCALE).reshape(CT, P).T),
            "bk": np.ascontiguousarray(
                bqkv[1 * E:2 * E][g].reshape(CT, P).T),
            "bv": bqkv[2 * E:3 * E][g].reshape(1, EG).copy(),
            "bp": (bproj if hg == 0 else np.zeros_like(bproj))
                  .reshape(1, E).copy(),
            "bmask": bmask,
            "ident": ident,
            "ones": np.ones((1, QCH), dtype=np.float32),
            "vones": np.ones((P, HPG), dtype=BF),
        })
    return in_maps


def kernel(x, Wqkv, bqkv, Wproj, bproj):
    with_bias = bool(
        np.any(np.asarray(bqkv)) or np.any(np.asarray(bproj)))
    nc = build_nc(with_bias)
    in_maps = make_in_maps(x, Wqkv, bqkv, Wproj, bproj)
    res = run_bass_kernel_spmd(nc, in_maps, list(range(8))).results
    out = np.zeros((B, S, E), dtype=np.float32)
    for c in range(8):
        out[c // 2] += res[c]["out"]
    return out


# revision 8
# speedup vs baseline: 1.7126x; 1.0716x over previous
"""Causal multi-head attention on 8 trn2 NeuronCores.

Sharding: core c -> (batch b = c//2, head-group hg = c%2).
Each head-group owns 8 of the 16 heads (512 of the 1024 embed dims after
the head split).

v3 layout (all matmul operands bf16, PSUM accumulation fp32):
  - qT, kT = (x[b] @ Wq_hg)^T, (x[b] @ Wk_hg)^T    [cols, rows] bf16
    (softmax 1/sqrt(d) scale folded into Wq on host)
  - v packed as va [rows, 8*(64+1)] bf16 with a ones column per head so
    the attn@V matmul also produces the softmax denominator (row 64).
  - scoresT [k, q] per (head, 512-q-chunk, 128-k-tile); exp -> bf16 on
    the Act engine; causal-diagonal tiles then have their first 128
    columns multiplied by a binary mask on the DVE (fast 2-byte mode).
  - normalize: denom row -> SBUF -> reciprocal_approx_fast (DVE) ->
    partition_broadcast (GPSIMD) -> one DVE multiply into ctxT bf16.
  - partial = ctxT.T @ Wproj_hg (+ bproj on hg==0 cores).
Host: out[b] = partial(b,0) + partial(b,1).

Emission order is software-pipelined: score matmuls run 3 k-tiles ahead
of the attn@V matmuls (so PE never waits on exp), the q/k projections
for later head-groups are interleaved into the attention stream (keeps
the PE HAM clock-gate at full speed), and DMA traffic is spread over
the sync/scalar/gpsimd queues so the lead-in is not serialized on one
engine.
"""

import sys

try:
    import concourse.bass as bass  # noqa: F401
except Exception:
    sys.path.insert(0, "/opt/trn_rl_repo")

import ml_dtypes
import numpy as np

import concourse.bass as bass
import concourse.mybir as mybir
import concourse.tile as tile
from concourse import bacc
from concourse.bass_utils import run_bass_kernel_spmd

F32 = mybir.dt.float32
F32R = mybir.dt.float32r
BF16 = mybir.dt.bfloat16
AF = mybir.ActivationFunctionType
BF = ml_dtypes.bfloat16

B, S, E = 4, 1024, 1024
H, D = 16, 64
HG = 2              # head groups (cores per batch)
HPG = H // HG       # 8 heads per group
EG = HPG * D        # 512 embed cols per group
P = 128
ET = E // P         # 8 embed tiles
RT = S // P         # 8 row tiles
CT = EG // P        # 4 col tiles of the group's q/k
QCH = 512           # q-chunk (moving free dim; ISA max for fp32 PSUM out)
NQC = S // QCH      # 2 q chunks
KTQ = QCH // P      # 4 k-tiles per q chunk
SCALE = 1.0 / np.sqrt(D)


def _emit(nc, tc, with_bias):
    xT = nc.dram_tensor("xT", [E, S], BF16, kind="ExternalInput")
    wq = nc.dram_tensor("wq", [E, EG], BF16, kind="ExternalInput")
    wk = nc.dram_tensor("wk", [E, EG], BF16, kind="ExternalInput")
    wv = nc.dram_tensor("wv", [E, EG], BF16, kind="ExternalInput")
    wp = nc.dram_tensor("wp", [EG, E], BF16, kind="ExternalInput")
    # packed constants: cb = binary causal mask(128) | vones(8)  (bf16)
    cb = nc.dram_tensor("cb", [P, P + HPG], BF16, kind="ExternalInput")
    # bqk = bq(4) | bk(4)  (f32, per-partition bias)
    bqk = nc.dram_tensor("bqk", [P, 2 * CT], F32, kind="ExternalInput")
    # crow = ones(512) | bv(512) | bp(1024)  (f32 rows)
    crow = nc.dram_tensor("crow", [1, QCH + EG + E], F32,
                          kind="ExternalInput")
    out = nc.dram_tensor("out", [S, E], F32, kind="ExternalOutput")

    with (
        tc.tile_pool(name="big", bufs=1) as p_big,
        tc.tile_pool(name="exs", bufs=6) as p_ex,
        tc.tile_pool(name="rc", bufs=4) as p_rc,
        tc.tile_pool(name="rcb", bufs=4) as p_rcb,
        tc.tile_pool(name="osb", bufs=4) as p_osb,
        tc.tile_pool(name="sm", bufs=1) as p_sm,
        tc.tile_pool(name="sc", bufs=3, space="PSUM") as p_sc,
        tc.tile_pool(name="qk", bufs=2, space="PSUM") as p_qk,
        tc.tile_pool(name="avp", bufs=3, space="PSUM") as p_av,
    ):
        # ---- constants: 3 packed DMAs on the gpsimd queue ----
        cb_sb = p_sm.tile([P, P + HPG], BF16, tag="cb", name="cbt")
        nc.gpsimd.dma_start(cb_sb[:], cb[:])
        mask_sb = cb_sb[:, 0:P]
        vones_sb = cb_sb[:, P:P + HPG]
        bqk_sb = p_sm.tile([P, 2 * CT], F32, tag="bqk", name="bqkt")
        nc.gpsimd.dma_start(bqk_sb[:], bqk[:])
        bq_sb = bqk_sb[:, 0:CT]
        bk_sb = bqk_sb[:, CT:2 * CT]
        crow_sb = p_sm.tile([1, QCH + EG + E], F32, tag="crow", name="crowt")
        nc.gpsimd.dma_start(crow_sb[:], crow[:])
        ones_sb = crow_sb[:, 0:QCH].bitcast(F32R)
        bv_sb = crow_sb[:, QCH:QCH + EG].bitcast(F32R)
        bp_sb = crow_sb[:, QCH + EG:].bitcast(F32R)

        # ---- persistent sbuf tiles ----
        xt_t = [p_big.tile([P, S], BF16, tag=f"xt{et}", name=f"xt{et}")
                for et in range(ET)]
        wq_t = [p_big.tile([P, EG], BF16, tag=f"wq{et}", name=f"wq{et}")
                for et in range(ET)]
        wk_t = [p_big.tile([P, EG], BF16, tag=f"wk{et}", name=f"wk{et}")
                for et in range(ET)]
        wv_t = [p_big.tile([P, EG], BF16, tag=f"wv{et}", name=f"wv{et}")
                for et in range(ET)]
        wp_t = [p_big.tile([P, E], BF16, tag=f"wp{et}", name=f"wp{et}")
                for et in range(CT)]
        qT_t = [p_big.tile([P, S], BF16, tag=f"qt{ct}", name=f"qt{ct}")
                for ct in range(CT)]
        kT_t = [p_big.tile([P, S], BF16, tag=f"kt{ct}", name=f"kt{ct}")
                for ct in range(CT)]
        va_t = [p_big.tile([P, HPG * (D + 1)], BF16, tag=f"va{rt}",
                           name=f"va{rt}") for rt in range(RT)]
        ctx_t = [p_big.tile([P, S], BF16, tag=f"cx{ct}", name=f"cx{ct}")
                 for ct in range(CT)]

        # ---- input DMA spread across three queues, ordered by use ----
        # sync:   xt first halves, xt second halves, wp
        # scalar: wq (drains before the first q convert needs the engine)
        # gpsimd: consts (above), wk, wv
        for et in range(ET):
            nc.sync.dma_start(xt_t[et][:, 0:QCH],
                              xT[et * P:(et + 1) * P, 0:QCH])
            nc.scalar.dma_start(wq_t[et][:], wq[et * P:(et + 1) * P, :])
            nc.gpsimd.dma_start(wk_t[et][:], wk[et * P:(et + 1) * P, :])
        for et in range(ET):
            nc.sync.dma_start(xt_t[et][:, QCH:S],
                              xT[et * P:(et + 1) * P, QCH:S])
            nc.gpsimd.dma_start(wv_t[et][:], wv[et * P:(et + 1) * P, :])
        for et in range(CT):
            nc.sync.dma_start(wp_t[et][:], wp[et * P:(et + 1) * P, :])

        # ---- q/k projection chunk: qT/kT[ct][:, rc*QCH:+QCH] ----
        def emit_qk_chunk(dst, w_t, b_sb, ct, rc, engine):
            ps = p_qk.tile([P, QCH], F32, tag="qk", name="qk")
            for et in range(ET):
                nc.tensor.matmul(
                    ps[:],
                    w_t[et][:, ct * P:(ct + 1) * P],
                    xt_t[et][:, rc * QCH:(rc + 1) * QCH],
                    start=(et == 0), stop=(et == ET - 1),
                )
            dst_ap = dst[ct][:, rc * QCH:(rc + 1) * QCH]
            if with_bias:
                nc.scalar.activation(
                    dst_ap, ps[:], AF.Identity, bias=b_sb[:, ct:ct + 1])
            elif engine == "act":
                nc.scalar.activation(dst_ap, ps[:], AF.Copy)
            else:
                nc.vector.tensor_copy(dst_ap, ps[:])

        # ---- v projection + augmented-va pack for one row tile ----
        def emit_v_rt(rt):
            va3 = va_t[rt][:].rearrange("p (h d) -> p h d", h=HPG)
            nc.vector.tensor_copy(
                va3[:, :, D:D + 1],
                vones_sb.rearrange("p (h o) -> p h o", o=1))
            ps = p_qk.tile([P, QCH], F32, tag="qk", name="qk")
            for et in range(ET):
                nc.tensor.matmul(
                    ps[:, 0:EG],
                    xt_t[et][:, rt * P:(rt + 1) * P],
                    wv_t[et][:],
                    start=(et == 0),
                    stop=(et == ET - 1 and not with_bias),
                )
            if with_bias:
                nc.tensor.matmul(
                    ps[:, 0:EG], ones_sb[0:1, 0:P], bv_sb[0:1, :],
                    start=False, stop=True,
                )
            ps3 = ps[:, 0:EG].rearrange("p (h d) -> p h d", h=HPG)
            nc.scalar.activation(va3[:, :, 0:D], ps3[:], AF.Copy)

        # ---- lead phase: q-ct0, k-ct0, v ----
        emit_qk_chunk(qT_t, wq_t, bq_sb, 0, 0, "act")
        emit_qk_chunk(qT_t, wq_t, bq_sb, 0, 1, "act")
        emit_qk_chunk(kT_t, wk_t, bk_sb, 0, 0, "act")
        emit_qk_chunk(kT_t, wk_t, bk_sb, 0, 1, "act")
        for rt in range(RT):
            emit_v_rt(rt)

        # remaining q/k chunks, injected mid-attention: dense full-K PE
        # work that keeps the HAM clock-gate warm while the queued exps
        # keep the Act engine busy
        inject = []
        for ct in range(1, CT):
            inject.append((qT_t, wq_t, bq_sb, ct, 0))
            inject.append((kT_t, wk_t, bk_sb, ct, 0))
            inject.append((qT_t, wq_t, bq_sb, ct, 1))
            inject.append((kT_t, wk_t, bk_sb, ct, 1))
        inject.reverse()

        # ---- attention ----
        for h in range(HPG):
            hp, hb = h // 2, (h % 2) * D
            va3s = [va_t[kt][:].rearrange("p (h d) -> p h d", h=HPG)[:, h, :]
                    for kt in range(RT)]
            for qc in range(NQC):
                n_kt = (qc + 1) * KTQ
                av = p_av.tile([D + 1, QCH], F32, tag="av", name="av")
                exs = {}

                def emit_sc(kt, qc=qc, exs=exs):
                    off = max(0, kt - qc * KTQ) * P
                    n = QCH - off
                    diag = (qc == 0) or (kt >= KTQ)
                    sc = p_sc.tile([P, QCH], F32, tag="sc", name="sc")
                    nc.tensor.matmul(
                        sc[:, 0:n],
                        kT_t[hp][hb:hb + D, kt * P:(kt + 1) * P],
                        qT_t[hp][hb:hb + D,
                                 qc * QCH + off:(qc + 1) * QCH],
                        start=True, stop=True,
                        tile_position=(hb, 0),
                    )
                    ex = p_ex.tile([P, QCH], BF16, tag="ex", name="ex")
                    nc.scalar.activation(ex[:, 0:n], sc[:, 0:n], AF.Exp)
                    if diag:
                        nc.vector.tensor_mul(
                            ex[:, 0:P], ex[:, 0:P], mask_sb)
                    exs[kt] = (ex, off, n)

                LOOK = 3
                for kt in range(min(LOOK, n_kt)):
                    emit_sc(kt)
                # inject one q/k projection chunk mid-unit: the queued
                # exps keep the Act engine busy while PE runs it
                if inject:
                    emit_qk_chunk(*inject.pop(), "dve")
                for kt in range(n_kt):
                    ex, off, n = exs.pop(kt)
                    nc.tensor.matmul(
                        av[:, off:QCH],
                        va3s[kt],
                        ex[:, 0:n],
                        start=(kt == 0), stop=(kt == n_kt - 1),
                    )
                    if kt + LOOK < n_kt:
                        emit_sc(kt + LOOK)

                # normalize: all off the PE stream
                # (reciprocal_approx_fast's bitwise seed misreads PSUM,
                # so stage the denominator row through SBUF first)
                dn_sb = p_rc.tile([1, QCH], F32, tag="dn", name="dn")
                nc.vector.tensor_copy(dn_sb[:], av[D:D + 1, :])
                rc_sb = p_rc.tile([1, QCH], F32, tag="rc", name="rc")
                nc.vector.reciprocal_approx_fast(rc_sb[:], dn_sb[:])
                rcb = p_rcb.tile([D, QCH], F32, tag="rcb", name="rcb")
                nc.gpsimd.partition_broadcast(rcb[:], rc_sb[:], channels=D)
                nc.vector.tensor_mul(
                    ctx_t[hp][hb:hb + D, qc * QCH:(qc + 1) * QCH],
                    av[0:D, :], rcb[:])

        # ---- output projection: partial = ctxT.T @ wp (+ bp) ----
        osb_eng = 0
        for rt in range(RT):
            for cc in range(E // QCH):
                ps = p_sc.tile([P, QCH], F32, tag="sc", name="sc")
                for et in range(CT):
                    nc.tensor.matmul(
                        ps[:],
                        ctx_t[et][:, rt * P:(rt + 1) * P],
                        wp_t[et][:, cc * QCH:(cc + 1) * QCH],
                        start=(et == 0),
                        stop=(et == CT - 1 and not with_bias),
                    )
                if with_bias:
                    nc.tensor.matmul(
                        ps[:], ones_sb[0:1, 0:P],
                        bp_sb[0:1, cc * QCH:(cc + 1) * QCH],
                        start=False, stop=True,
                    )
                osb = p_osb.tile([P, QCH], F32, tag="osb", name="osb")
                if osb_eng == 0:
                    nc.vector.tensor_copy(osb[:], ps[:])
                else:
                    nc.scalar.activation(osb[:], ps[:], AF.Copy)
                osb_eng = (osb_eng + 1) % 2
                dma_eng = nc.sync if cc == 0 else nc.gpsimd
                dma_eng.dma_start(
                    out[rt * P:(rt + 1) * P, cc * QCH:(cc + 1) * QCH],
                    osb[:])


def build_nc(with_bias=False):
    nc = bacc.Bacc("TRN2", target_bir_lowering=False, debug=False)
    with tile.TileContext(nc) as tc, nc.allow_low_precision(
        reason="bf16 matmul pipeline; fp32 PSUM accumulate"
    ):
        _emit(nc, tc, with_bias)
    nc.compile()
    return nc


def make_in_maps(x, Wqkv, bqkv, Wproj, bproj):
    x = np.asarray(x, dtype=np.float32)
    Wqkv = np.asarray(Wqkv, dtype=np.float32)
    bqkv = np.asarray(bqkv, dtype=np.float32)
    Wproj = np.asarray(Wproj, dtype=np.float32)
    bproj = np.asarray(bproj, dtype=np.float32)
    keep = np.triu(np.ones((P, P), dtype=np.float32))  # [k, q]: k <= q
    cb = np.concatenate([
        keep,                                 # binary causal mask
        np.ones((P, HPG), dtype=np.float32),  # vones
    ], axis=1).astype(BF)
    in_maps = []
    for c in range(8):
        b, hg = c // 2, c % 2
        g = slice(hg * EG, (hg + 1) * EG)
        bqk = np.concatenate([
            (bqkv[0 * E:1 * E][g] * SCALE).reshape(CT, P).T,
            bqkv[1 * E:2 * E][g].reshape(CT, P).T], axis=1)
        crow = np.concatenate([
            np.ones(QCH, dtype=np.float32),
            bqkv[2 * E:3 * E][g],
            bproj if hg == 0 else np.zeros_like(bproj),
        ]).reshape(1, QCH + EG + E)
        in_maps.append({
            "xT": np.ascontiguousarray(x[b].T).astype(BF),
            "wq": np.ascontiguousarray(
                Wqkv[:, 0 * E:1 * E][:, g] * SCALE).astype(BF),
            "wk": np.ascontiguousarray(Wqkv[:, 1 * E:2 * E][:, g]).astype(BF),
            "wv": np.ascontiguousarray(Wqkv[:, 2 * E:3 * E][:, g]).astype(BF),
            "wp": np.ascontiguousarray(Wproj[g, :]).astype(BF),
            "cb": cb,
            "bqk": np.ascontiguousarray(bqk),
            "crow": np.ascontiguousarray(crow),
        })
    return in_maps


def kernel(x, Wqkv, bqkv, Wproj, bproj):
    with_bias = bool(
        np.any(np.asarray(bqkv)) or np.any(np.asarray(bproj)))
    nc = build_nc(with_bias)
    in_maps = make_in_maps(x, Wqkv, bqkv, Wproj, bproj)
    res = run_bass_kernel_spmd(nc, in_maps, list(range(8))).results
    out = np.zeros((B, S, E), dtype=np.float32)
    for c in range(8):
        out[c // 2] += res[c]["out"]
    return out


# revision 9
# speedup vs baseline: 1.7154x; 1.0017x over previous
"""Causal multi-head attention on 8 trn2 NeuronCores.

Sharding: core c -> (batch b = c//2, head-group hg = c%2).
Each head-group owns 8 of the 16 heads (512 of the 1024 embed dims after
the head split).

v3 layout (all matmul operands bf16, PSUM accumulation fp32):
  - qT, kT = (x[b] @ Wq_hg)^T, (x[b] @ Wk_hg)^T    [cols, rows] bf16
    (softmax 1/sqrt(d) scale folded into Wq on host)
  - v packed as va [rows, 8*(64+1)] bf16 with a ones column per head so
    the attn@V matmul also produces the softmax denominator (row 64).
  - scoresT [k, q] per (head, 512-q-chunk, 128-k-tile); exp -> bf16 on
    the Act engine; causal-diagonal tiles then have their first 128
    columns multiplied by a binary mask on the DVE (fast 2-byte mode).
  - normalize: denom row -> SBUF -> reciprocal_approx_fast (DVE) ->
    partition_broadcast (GPSIMD) -> one DVE multiply into ctxT bf16.
  - partial = ctxT.T @ Wproj_hg (+ bproj on hg==0 cores).
Host: out[b] = partial(b,0) + partial(b,1).

Emission order is software-pipelined: score matmuls run 3 k-tiles ahead
of the attn@V matmuls (so PE never waits on exp), the q/k projections
for later head-groups are interleaved into the attention stream (keeps
the PE HAM clock-gate at full speed), and DMA traffic is spread over
the sync/scalar/gpsimd queues so the lead-in is not serialized on one
engine.
"""

import sys

try:
    import concourse.bass as bass  # noqa: F401
except Exception:
    sys.path.insert(0, "/opt/trn_rl_repo")

import ml_dtypes
import numpy as np

import concourse.bass as bass
import concourse.mybir as mybir
import concourse.tile as tile
from concourse import bacc
from concourse.bass_utils import run_bass_kernel_spmd

F32 = mybir.dt.float32
F32R = mybir.dt.float32r
BF16 = mybir.dt.bfloat16
AF = mybir.ActivationFunctionType
BF = ml_dtypes.bfloat16

B, S, E = 4, 1024, 1024
H, D = 16, 64
HG = 2              # head groups (cores per batch)
HPG = H // HG       # 8 heads per group
EG = HPG * D        # 512 embed cols per group
P = 128
ET = E // P         # 8 embed tiles
RT = S // P         # 8 row tiles
CT = EG // P        # 4 col tiles of the group's q/k
QCH = 512           # q-chunk (moving free dim; ISA max for fp32 PSUM out)
NQC = S // QCH      # 2 q chunks
KTQ = QCH // P      # 4 k-tiles per q chunk
SCALE = 1.0 / np.sqrt(D)


def _emit(nc, tc, with_bias):
    xT = nc.dram_tensor("xT", [E, S], BF16, kind="ExternalInput")
    wq = nc.dram_tensor("wq", [E, EG], BF16, kind="ExternalInput")
    wk = nc.dram_tensor("wk", [E, EG], BF16, kind="ExternalInput")
    wv = nc.dram_tensor("wv", [E, EG], BF16, kind="ExternalInput")
    wp = nc.dram_tensor("wp", [EG, E], BF16, kind="ExternalInput")
    # packed constants: cb = binary causal mask(128) | vones(8)  (bf16)
    cb = nc.dram_tensor("cb", [P, P + HPG], BF16, kind="ExternalInput")
    # bqk = bq(4) | bk(4)  (f32, per-partition bias)
    bqk = nc.dram_tensor("bqk", [P, 2 * CT], F32, kind="ExternalInput")
    # crow = ones(512) | bv(512) | bp(1024)  (f32 rows)
    crow = nc.dram_tensor("crow", [1, QCH + EG + E], F32,
                          kind="ExternalInput")
    out = nc.dram_tensor("out", [S, E], F32, kind="ExternalOutput")

    with (
        tc.tile_pool(name="big", bufs=1) as p_big,
        tc.tile_pool(name="exs", bufs=6) as p_ex,
        tc.tile_pool(name="rc", bufs=4) as p_rc,
        tc.tile_pool(name="rcb", bufs=4) as p_rcb,
        tc.tile_pool(name="osb", bufs=4) as p_osb,
        tc.tile_pool(name="sm", bufs=1) as p_sm,
        tc.tile_pool(name="sc", bufs=3, space="PSUM") as p_sc,
        tc.tile_pool(name="qk", bufs=2, space="PSUM") as p_qk,
        tc.tile_pool(name="avp", bufs=3, space="PSUM") as p_av,
    ):
        # ---- constants: 3 packed DMAs on the gpsimd queue ----
        cb_sb = p_sm.tile([P, P + HPG], BF16, tag="cb", name="cbt")
        nc.gpsimd.dma_start(cb_sb[:], cb[:])
        mask_sb = cb_sb[:, 0:P]
        vones_sb = cb_sb[:, P:P + HPG]
        bqk_sb = p_sm.tile([P, 2 * CT], F32, tag="bqk", name="bqkt")
        nc.gpsimd.dma_start(bqk_sb[:], bqk[:])
        bq_sb = bqk_sb[:, 0:CT]
        bk_sb = bqk_sb[:, CT:2 * CT]
        crow_sb = p_sm.tile([1, QCH + EG + E], F32, tag="crow", name="crowt")
        nc.gpsimd.dma_start(crow_sb[:], crow[:])
        ones_sb = crow_sb[:, 0:QCH].bitcast(F32R)
        bv_sb = crow_sb[:, QCH:QCH + EG].bitcast(F32R)
        bp_sb = crow_sb[:, QCH + EG:].bitcast(F32R)

        # ---- persistent sbuf tiles ----
        xt_t = [p_big.tile([P, S], BF16, tag=f"xt{et}", name=f"xt{et}")
                for et in range(ET)]
        wq_t = [p_big.tile([P, EG], BF16, tag=f"wq{et}", name=f"wq{et}")
                for et in range(ET)]
        wk_t = [p_big.tile([P, EG], BF16, tag=f"wk{et}", name=f"wk{et}")
                for et in range(ET)]
        wv_t = [p_big.tile([P, EG], BF16, tag=f"wv{et}", name=f"wv{et}")
                for et in range(ET)]
        wp_t = [p_big.tile([P, E], BF16, tag=f"wp{et}", name=f"wp{et}")
                for et in range(CT)]
        qT_t = [p_big.tile([P, S], BF16, tag=f"qt{ct}", name=f"qt{ct}")
                for ct in range(CT)]
        kT_t = [p_big.tile([P, S], BF16, tag=f"kt{ct}", name=f"kt{ct}")
                for ct in range(CT)]
        va_t = [p_big.tile([P, HPG * (D + 1)], BF16, tag=f"va{rt}",
                           name=f"va{rt}") for rt in range(RT)]
        ctx_t = [p_big.tile([P, S], BF16, tag=f"cx{ct}", name=f"cx{ct}")
                 for ct in range(CT)]

        # ---- input DMA spread across three queues, ordered by use ----
        # DMA cost is dominated by the ~650ns per-instruction issue on
        # the owning engine, so fewer+bigger transfers win.
        # sync:   xt full tiles, wv second half, wp
        # scalar: wq, wv first half (drains before the first q convert)
        # gpsimd: consts (above), wk
        for et in range(ET):
            nc.sync.dma_start(xt_t[et][:], xT[et * P:(et + 1) * P, :])
            nc.scalar.dma_start(wq_t[et][:], wq[et * P:(et + 1) * P, :])
            nc.gpsimd.dma_start(wk_t[et][:], wk[et * P:(et + 1) * P, :])
        for et in range(ET):
            eng = nc.scalar if et < 4 else nc.sync
            eng.dma_start(wv_t[et][:], wv[et * P:(et + 1) * P, :])
        for et in range(CT):
            nc.sync.dma_start(wp_t[et][:], wp[et * P:(et + 1) * P, :])

        # ---- q/k projection chunk: qT/kT[ct][:, rc*QCH:+QCH] ----
        def emit_qk_chunk(dst, w_t, b_sb, ct, rc, engine):
            ps = p_qk.tile([P, QCH], F32, tag="qk", name="qk")
            for et in range(ET):
                nc.tensor.matmul(
                    ps[:],
                    w_t[et][:, ct * P:(ct + 1) * P],
                    xt_t[et][:, rc * QCH:(rc + 1) * QCH],
                    start=(et == 0), stop=(et == ET - 1),
                )
            dst_ap = dst[ct][:, rc * QCH:(rc + 1) * QCH]
            if with_bias:
                nc.scalar.activation(
                    dst_ap, ps[:], AF.Identity, bias=b_sb[:, ct:ct + 1])
            elif engine == "act":
                nc.scalar.activation(dst_ap, ps[:], AF.Copy)
            else:
                nc.vector.tensor_copy(dst_ap, ps[:])

        # ---- v projection + augmented-va pack for one row tile ----
        def emit_v_rt(rt):
            va3 = va_t[rt][:].rearrange("p (h d) -> p h d", h=HPG)
            nc.vector.tensor_copy(
                va3[:, :, D:D + 1],
                vones_sb.rearrange("p (h o) -> p h o", o=1))
            ps = p_qk.tile([P, QCH], F32, tag="qk", name="qk")
            for et in range(ET):
                nc.tensor.matmul(
                    ps[:, 0:EG],
                    xt_t[et][:, rt * P:(rt + 1) * P],
                    wv_t[et][:],
                    start=(et == 0),
                    stop=(et == ET - 1 and not with_bias),
                )
            if with_bias:
                nc.tensor.matmul(
                    ps[:, 0:EG], ones_sb[0:1, 0:P], bv_sb[0:1, :],
                    start=False, stop=True,
                )
            ps3 = ps[:, 0:EG].rearrange("p (h d) -> p h d", h=HPG)
            nc.scalar.activation(va3[:, :, 0:D], ps3[:], AF.Copy)

        # ---- lead phase: q-ct0, k-ct0, v ----
        emit_qk_chunk(qT_t, wq_t, bq_sb, 0, 0, "act")
        emit_qk_chunk(qT_t, wq_t, bq_sb, 0, 1, "act")
        emit_qk_chunk(kT_t, wk_t, bk_sb, 0, 0, "act")
        emit_qk_chunk(kT_t, wk_t, bk_sb, 0, 1, "act")
        for rt in range(RT):
            emit_v_rt(rt)

        # remaining q/k chunks, injected mid-attention: dense full-K PE
        # work that keeps the HAM clock-gate warm while the queued exps
        # keep the Act engine busy
        inject = []
        for ct in range(1, CT):
            inject.append((qT_t, wq_t, bq_sb, ct, 0))
            inject.append((kT_t, wk_t, bk_sb, ct, 0))
            inject.append((qT_t, wq_t, bq_sb, ct, 1))
            inject.append((kT_t, wk_t, bk_sb, ct, 1))
        inject.reverse()

        # ---- attention ----
        for h in range(HPG):
            hp, hb = h // 2, (h % 2) * D
            va3s = [va_t[kt][:].rearrange("p (h d) -> p h d", h=HPG)[:, h, :]
                    for kt in range(RT)]
            for qc in range(NQC):
                n_kt = (qc + 1) * KTQ
                av = p_av.tile([D + 1, QCH], F32, tag="av", name="av")
                exs = {}

                def emit_sc(kt, qc=qc, exs=exs):
                    off = max(0, kt - qc * KTQ) * P
                    n = QCH - off
                    diag = (qc == 0) or (kt >= KTQ)
                    sc = p_sc.tile([P, QCH], F32, tag="sc", name="sc")
                    nc.tensor.matmul(
                        sc[:, 0:n],
                        kT_t[hp][hb:hb + D, kt * P:(kt + 1) * P],
                        qT_t[hp][hb:hb + D,
                                 qc * QCH + off:(qc + 1) * QCH],
                        start=True, stop=True,
                        tile_position=(hb, 0),
                    )
                    ex = p_ex.tile([P, QCH], BF16, tag="ex", name="ex")
                    nc.scalar.activation(ex[:, 0:n], sc[:, 0:n], AF.Exp)
                    if diag:
                        nc.vector.tensor_mul(
                            ex[:, 0:P], ex[:, 0:P], mask_sb)
                    exs[kt] = (ex, off, n)

                LOOK = 3
                for kt in range(min(LOOK, n_kt)):
                    emit_sc(kt)
                # inject one q/k projection chunk mid-unit: the queued
                # exps keep the Act engine busy while PE runs it
                if inject:
                    emit_qk_chunk(*inject.pop(), "dve")
                for kt in range(n_kt):
                    ex, off, n = exs.pop(kt)
                    nc.tensor.matmul(
                        av[:, off:QCH],
                        va3s[kt],
                        ex[:, 0:n],
                        start=(kt == 0), stop=(kt == n_kt - 1),
                    )
                    if kt + LOOK < n_kt:
                        emit_sc(kt + LOOK)

                # normalize: all off the PE stream
                # (reciprocal_approx_fast's bitwise seed misreads PSUM,
                # so stage the denominator row through SBUF first)
                dn_sb = p_rc.tile([1, QCH], F32, tag="dn", name="dn")
                nc.vector.tensor_copy(dn_sb[:], av[D:D + 1, :])
                rc_sb = p_rc.tile([1, QCH], F32, tag="rc", name="rc")
                nc.vector.reciprocal_approx_fast(rc_sb[:], dn_sb[:])
                rcb = p_rcb.tile([D, QCH], F32, tag="rcb", name="rcb")
                nc.gpsimd.partition_broadcast(rcb[:], rc_sb[:], channels=D)
                nc.vector.tensor_mul(
                    ctx_t[hp][hb:hb + D, qc * QCH:(qc + 1) * QCH],
                    av[0:D, :], rcb[:])

        # ---- output projection: partial = ctxT.T @ wp (+ bp) ----
        osb_eng = 0
        for rt in range(RT):
            for cc in range(E // QCH):
                ps = p_sc.tile([P, QCH], F32, tag="sc", name="sc")
                for et in range(CT):
                    nc.tensor.matmul(
                        ps[:],
                        ctx_t[et][:, rt * P:(rt + 1) * P],
                        wp_t[et][:, cc * QCH:(cc + 1) * QCH],
                        start=(et == 0),
                        stop=(et == CT - 1 and not with_bias),
                    )
                if with_bias:
                    nc.tensor.matmul(
                        ps[:], ones_sb[0:1, 0:P],
                        bp_sb[0:1, cc * QCH:(cc + 1) * QCH],
                        start=False, stop=True,
                    )
                osb = p_osb.tile([P, QCH], F32, tag="osb", name="osb")
                if osb_eng == 0:
                    nc.vector.tensor_copy(osb[:], ps[:])
                else:
                    nc.scalar.activation(osb[:], ps[:], AF.Copy)
                osb_eng = (osb_eng + 1) % 2
                dma_eng = nc.sync if cc == 0 else nc.gpsimd
                dma_eng.dma_start(
                    out[rt * P:(rt + 1) * P, cc * QCH:(cc + 1) * QCH],
                    osb[:])


def build_nc(with_bias=False):
    nc = bacc.Bacc("TRN2", target_bir_lowering=False, debug=False)
    with tile.TileContext(nc) as tc, nc.allow_low_precision(
        reason="bf16 matmul pipeline; fp32 PSUM accumulate"
    ):
        _emit(nc, tc, with_bias)
    nc.compile()
    return nc


def make_in_maps(x, Wqkv, bqkv, Wproj, bproj):
    x = np.asarray(x, dtype=np.float32)
    Wqkv = np.asarray(Wqkv, dtype=np.float32)
    bqkv = np.asarray(bqkv, dtype=np.float32)
    Wproj = np.asarray(Wproj, dtype=np.float32)
    bproj = np.asarray(bproj, dtype=np.float32)
    keep = np.triu(np.ones((P, P), dtype=np.float32))  # [k, q]: k <= q
    cb = np.concatenate([
        keep,                                 # binary causal mask
        np.ones((P, HPG), dtype=np.float32),  # vones
    ], axis=1).astype(BF)
    in_maps = []
    for c in range(8):
        b, hg = c // 2, c % 2
        g = slice(hg * EG, (hg + 1) * EG)
        bqk = np.concatenate([
            (bqkv[0 * E:1 * E][g] * SCALE).reshape(CT, P).T,
            bqkv[1 * E:2 * E][g].reshape(CT, P).T], axis=1)
        crow = np.concatenate([
            np.ones(QCH, dtype=np.float32),
            bqkv[2 * E:3 * E][g],
            bproj if hg == 0 else np.zeros_like(bproj),
        ]).reshape(1, QCH + EG + E)
        in_maps.append({
            "xT": np.ascontiguousarray(x[b].T).astype(BF),
            "wq": np.ascontiguousarray(
                Wqkv[:, 0 * E:1 * E][:, g] * SCALE).astype(BF),
            "wk": np.ascontiguousarray(Wqkv[:, 1 * E:2 * E][:, g]).astype(BF),
            "wv": np.ascontiguousarray(Wqkv[:, 2 * E:3 * E][:, g]).astype(BF),
            "wp": np.ascontiguousarray(Wproj[g, :]).astype(BF),
            "cb": cb,
            "bqk": np.ascontiguousarray(bqk),
            "crow": np.ascontiguousarray(crow),
        })
    return in_maps


def kernel(x, Wqkv, bqkv, Wproj, bproj):
    with_bias = bool(
        np.any(np.asarray(bqkv)) or np.any(np.asarray(bproj)))
    nc = build_nc(with_bias)
    in_maps = make_in_maps(x, Wqkv, bqkv, Wproj, bproj)
    res = run_bass_kernel_spmd(nc, in_maps, list(range(8))).results
    out = np.zeros((B, S, E), dtype=np.float32)
    for c in range(8):
        out[c // 2] += res[c]["out"]
    return out
